# revision 24
# baseline (speedup 1.0000x reference)
"""GAT layer (AdaptiveBreadthLayer) on 8 TRN2 NeuronCores.

Strategy:
  - dst-shard: core c owns destination nodes [c*6272, (c+1)*6272) (N padded
    50000 -> 50176). Every edge lives on exactly one core (by dst), so no
    cross-core reduction and no collectives are needed.
  - Each core redundantly computes the full projection table
    row(n) = [feat(n) (256) | el(n) (4) | er(n) (4) | pad] in bf16
    (phase 1), stored in its local DRAM, split into two halves so rows are
    indexable with int16 for dma_gather.
  - Phase 2 walks the core's destination tiles (128 dst nodes each, load
    balanced by in-degree binning). Per tile: dma_gather of table rows for
    the tile's edges' sources, one-hot (edge -> dst-slot) matmuls for the
    segment softmax denominator and the weighted feature aggregation.
    Softmax is computed without max-subtraction (values are small,
    mathematically identical), and normalization by 1/denom is applied per
    destination after aggregation instead of per edge.
"""

import sys

import numpy as np

sys.path.insert(0, "/opt/trn_rl_repo")

import ml_dtypes

import concourse.bacc as bacc
import concourse.bass as bass
import concourse.mybir as mybir
from concourse.tile import TileContext

BF16 = mybir.dt.bfloat16
F32 = mybir.dt.float32
I32 = mybir.dt.int32
I16 = mybir.dt.int16

P = 128
H = 4
D = 64
HD = H * D  # 256
ROWP = 384  # padded table row: feat(256) | el(4) | er(4) | pad -> 768B
IN_DIM = 256
NEG_SLOPE = 0.2

N = 50000
E = 800000
NC = 8
N_PAD = 50176  # 8 * 49 * 128
NR = N_PAD // NC  # 6272 rows per core
TILES = NR // P  # 49 dst tiles per core
HALF = N_PAD // 2  # 25088 rows per table half (int16-indexable)
PAD_DSTLOC = 200.0
DBG_NO_ER = False
DBG_NO_GATHER = False
DBG_NO_ERDRAM = False
DBG_STAGE = 99
DBG_HOST_TABLE = False
DBG_NO_SMALL_DMA = False


# --------------------------------------------------------------------------
# host-side preprocessing (index structures only; no float math)
# --------------------------------------------------------------------------

def _prep_core(src_c, dst_c, base):
    """Bin a core's dst nodes into TILES bins of P nodes balanced by
    in-degree."""
    dst_local = dst_c - base
    indeg = np.bincount(dst_local, minlength=NR)
    order = np.argsort(-indeg, kind="stable")  # desc by degree
    rounds = order.reshape(P, TILES).copy()  # snake-fill P rounds x TILES bins
    rounds[1::2] = rounds[1::2, ::-1]
    members = rounds
    tile_of = np.empty(NR, dtype=np.int64)
    pos_of = np.empty(NR, dtype=np.int64)
    tile_of[members.ravel()] = np.tile(np.arange(TILES), P)
    pos_of[members.ravel()] = np.repeat(np.arange(P), TILES)

    counts = indeg[members].sum(axis=0)
    tile_order = np.argsort(-counts, kind="stable")
    rank_of_tile = np.empty(TILES, dtype=np.int64)
    rank_of_tile[tile_order] = np.arange(TILES)

    member_ids = members[:, tile_order] + base  # [P, TILES] global ids
    t_e = rank_of_tile[tile_of[dst_local]]
    p_e = pos_of[dst_local]
    return member_ids, t_e, p_e


def preprocess(src, dst):
    src = np.asarray(src).astype(np.int64)
    dst = np.asarray(dst).astype(np.int64)
    core_of = dst // NR
    per_core = []
    lo_counts = np.zeros((NC, TILES), dtype=np.int64)
    hi_counts = np.zeros((NC, TILES), dtype=np.int64)
    for c in range(NC):
        m = core_of == c
        member_ids, t_e, p_e = _prep_core(src[m], dst[m], c * NR)
        is_lo = src[m] < HALF
        per_core.append((src[m], member_ids, t_e, p_e, is_lo))
        np.add.at(lo_counts[c], t_e[is_lo], 1)
        np.add.at(hi_counts[c], t_e[~is_lo], 1)
    clo = np.ceil(lo_counts.max(axis=0) / P).astype(np.int64)
    chi = np.ceil(hi_counts.max(axis=0) / P).astype(np.int64)
    # keep at least one chunk total per tile
    both_zero = (clo + chi) == 0
    clo[both_zero] = 1
    c_tot = clo + chi
    sum_c = int(c_tot.sum())
    offs = np.concatenate([[0], np.cumsum(c_tot)[:-1]])

    aux = []
    for c in range(NC):
        src_c, member_ids, t_e, p_e, is_lo = per_core[c]
        idxw = np.zeros((P, sum_c * 8), dtype=np.int16)
        dstloc = np.full((P, sum_c), PAD_DSTLOC, dtype=ml_dtypes.bfloat16)
        for half, cnt_sched in ((True, clo), (False, chi)):
            sel = is_lo == half
            t_h = t_e[sel]
            s_h = src_c[sel] - (0 if half else HALF)
            p_h = p_e[sel]
            order = np.argsort(t_h, kind="stable")
            t_s, s_s, p_s = t_h[order], s_h[order], p_h[order]
            tile_starts = np.searchsorted(t_s, np.arange(TILES))
            q = np.arange(len(order)) - tile_starts[t_s]
            # chunk within this half's chunk block
            chunk = q // P
            slot = q % P
            col = offs[t_s] + (0 if half else clo[t_s]) + chunk
            dstloc[slot, col] = p_s.astype(ml_dtypes.bfloat16)
            # wrapped-16 idx layout: edge position i (= q) within the half
            # block maps to idx[(i % 16), i // 16] of that block
            blk0 = (offs[t_s] + (0 if half else clo[t_s])) * 8  # idx cols base
            icol = blk0 + q // 16
            irow = q % 16
            idxw[irow, icol] = s_s.astype(np.int16)
        # replicate across the 8 groups of 16 partitions
        for g in range(1, 8):
            idxw[g * 16 : (g + 1) * 16] = idxw[0:16]
        aux.append(
            dict(
                idxw=idxw,
                dstloc=dstloc,
                member_ids=np.ascontiguousarray(member_ids.astype(np.int32)),
            )
        )
    return aux, [int(x) for x in clo], [int(x) for x in chi]


# --------------------------------------------------------------------------
# device kernel builder
# --------------------------------------------------------------------------

def build_kernel(n_pad, tiles, clo, chi):
    c_tot = [a + b for a, b in zip(clo, chi)]
    sum_c = int(sum(c_tot))
    half = n_pad // 2
    nc = bacc.Bacc()

    hT = nc.declare_dram_parameter("hT", [IN_DIM, n_pad], BF16, isOutput=False)
    Wb = nc.declare_dram_parameter("Wb", [IN_DIM, HD], BF16, isOutput=False)
    WTb = nc.declare_dram_parameter("WTb", [IN_DIM, HD], BF16, isOutput=False)
    ALR = nc.declare_dram_parameter("ALR", [IN_DIM, 2 * H], BF16, isOutput=False)
    bias_rep = nc.declare_dram_parameter("bias_rep", [P, HD], F32, isOutput=False)
    iota_rep = nc.declare_dram_parameter("iota_rep", [P, P], BF16, isOutput=False)
    ident = nc.declare_dram_parameter("ident", [P, P], BF16, isOutput=False)
    idxw = nc.declare_dram_parameter("idxw", [P, sum_c * 8], I16, isOutput=False)
    dstloc = nc.declare_dram_parameter("dstloc", [P, sum_c], BF16, isOutput=False)
    member_ids = nc.declare_dram_parameter(
        "member_ids", [P, tiles], I32, isOutput=False
    )
    out = nc.declare_dram_parameter("out", [tiles * P, D], F32, isOutput=True)
    if DBG_HOST_TABLE:
        t_lo_p = nc.declare_dram_parameter("t_lo_in", [half, ROWP], BF16, isOutput=False)
        t_hi_p = nc.declare_dram_parameter("t_hi_in", [half, ROWP], BF16, isOutput=False)
        er_p = nc.declare_dram_parameter("er_in", [n_pad, H], BF16, isOutput=False)

    AL = mybir.AluOpType
    KCH = IN_DIM // P  # 2 contraction chunks

    with TileContext(nc) as tc:
        with (
            tc.tile_pool(name="const", bufs=1) as constp,
            tc.tile_pool(name="dram", bufs=1, space="DRAM") as dramp,
        ):
            if DBG_HOST_TABLE:
                t_lo, t_hi, er_dram = t_lo_p, t_hi_p, er_p
            else:
                t_lo = dramp.tile([half, ROWP], BF16)
                t_hi = dramp.tile([half, ROWP], BF16)
                er_dram = dramp.tile([n_pad, H], BF16)

            W_sb = constp.tile([P, KCH * HD], BF16)
            WT_sb = constp.tile([P, KCH * HD], BF16)
            ALR_sb = constp.tile([P, KCH * 2 * H], BF16)
            WALR_sb = constp.tile([P, KCH * 2 * H], BF16)
            bias_sb = constp.tile([P, HD], F32)
            iota_sb = constp.tile([P, P], BF16)
            ident_sb = constp.tile([P, P], BF16)
            for kk in range(KCH):
                nc.sync.dma_start(
                    out=W_sb[:, kk * HD : (kk + 1) * HD],
                    in_=Wb[kk * P : (kk + 1) * P, :],
                )
                nc.sync.dma_start(
                    out=WT_sb[:, kk * HD : (kk + 1) * HD],
                    in_=WTb[kk * P : (kk + 1) * P, :],
                )
                nc.sync.dma_start(
                    out=ALR_sb[:, kk * 2 * H : (kk + 1) * 2 * H],
                    in_=ALR[kk * P : (kk + 1) * P, :],
                )
            nc.sync.dma_start(out=bias_sb[:], in_=bias_rep[:, :])
            nc.sync.dma_start(out=iota_sb[:], in_=iota_rep[:, :])
            nc.sync.dma_start(out=ident_sb[:], in_=ident[:, :])

            # WALR = W @ ALR
            with tc.tile_pool(name="setup_ps", bufs=1, space="PSUM") as setupps:
                for ic in range(KCH):
                    walr_ps = setupps.tile([P, 2 * H], F32)
                    for kk in range(KCH):
                        nc.tensor.matmul(
                            walr_ps[:],
                            lhsT=WT_sb[:, kk * HD + ic * P : kk * HD + (ic + 1) * P],
                            rhs=ALR_sb[:, kk * 2 * H : (kk + 1) * 2 * H],
                            start=(kk == 0),
                            stop=(kk == KCH - 1),
                        )
                    nc.vector.tensor_copy(
                        out=WALR_sb[:, ic * 2 * H : (ic + 1) * 2 * H], in_=walr_ps[:]
                    )

            # ------------------- phase 1: projection table -------------------
            OB = 1024  # rows per outer block
            n_ob = (n_pad // OB) if not DBG_HOST_TABLE else 0
            with (
                tc.tile_pool(name="p1", bufs=3) as p1,
                tc.tile_pool(name="p1ps", bufs=2, space="PSUM") as p1ps,
            ):
                for ob in range(n_ob):
                    start = ob * OB
                    hT_t = p1.tile([P, KCH, OB], BF16, name="hT_t", tag="hT_t")
                    for kk in range(KCH):
                        nc.sync.dma_start(
                            out=hT_t[:, kk, :],
                            in_=hT[kk * P : (kk + 1) * P, start : start + OB],
                        )
                    for sub in range(OB // P):
                        feat_ps = p1ps.tile([P, HD], F32, name="feat_ps", tag="feat_ps")
                        elr_ps = p1ps.tile([P, 2 * H], F32, name="elr_ps", tag="elr_ps")
                        for kk in range(KCH):
                            lh = hT_t[:, kk, sub * P : (sub + 1) * P]
                            nc.tensor.matmul(
                                feat_ps[:],
                                lhsT=lh,
                                rhs=W_sb[:, kk * HD : (kk + 1) * HD],
                                start=(kk == 0),
                                stop=(kk == KCH - 1),
                            )
                            nc.tensor.matmul(
                                elr_ps[:],
                                lhsT=lh,
                                rhs=WALR_sb[:, kk * 2 * H : (kk + 1) * 2 * H],
                                start=(kk == 0),
                                stop=(kk == KCH - 1),
                            )
                        trow = p1.tile([P, ROWP], BF16, name="trow", tag="trow")
                        nc.vector.memset(trow[:, HD + 2 * H : ROWP], 0.0)
                        nc.any.tensor_copy(out=trow[:, 0:HD], in_=feat_ps[:])
                        nc.any.tensor_copy(out=trow[:, HD : HD + 2 * H], in_=elr_ps[:])
                        r0 = start + sub * P
                        tgt = t_lo if r0 < half else t_hi
                        rr = r0 if r0 < half else r0 - half
                        nc.sync.dma_start(out=tgt[rr : rr + P, :], in_=trow[:])
                        if not DBG_NO_ERDRAM:
                            nc.sync.dma_start(
                                out=er_dram[r0 : r0 + P, :],
                                in_=trow[:, HD + H : HD + 2 * H],
                            )

            # ------------------- phase 2: edge aggregation -------------------
            with (
                tc.tile_pool(name="p2", bufs=2) as p2,
                tc.tile_pool(name="p2s", bufs=3) as p2s,
                tc.tile_pool(name="outps", bufs=2, space="PSUM") as outps_pool,
                tc.tile_pool(name="denps", bufs=2, space="PSUM") as denps_pool,
                tc.tile_pool(name="ergps", bufs=2, space="PSUM") as ergps_pool,
                tc.tile_pool(name="sps", bufs=2, space="PSUM") as sps_pool,
            ):
                off = 0
                for t in range(tiles):
                    CL, CH = int(clo[t]), int(chi[t])
                    C = CL + CH
                    idx_t = p2.tile([P, C * 8], I16, name="idx_t", tag="idx")
                    nc.sync.dma_start(
                        out=idx_t[:], in_=idxw[:, off * 8 : (off + C) * 8]
                    )
                    dl_t = p2.tile([P, C], BF16, name="dl_t", tag="dl")
                    mem_t = p2.tile([P, 1], I32, name="mem_t", tag="mem")
                    if not DBG_NO_SMALL_DMA:
                        nc.sync.dma_start(out=dl_t[:], in_=dstloc[:, off : off + C])
                        nc.sync.dma_start(out=mem_t[:], in_=member_ids[:, t : t + 1])
                    else:
                        nc.vector.memset(dl_t[:], 0.0)
                        nc.vector.memset(mem_t[:].bitcast(F32), 0.0)

                    er_t = p2.tile([P, H], BF16, name="er_t", tag="er")
                    if DBG_NO_ER:
                        nc.vector.memset(er_t[:], 0.0)
                    else:
                        nc.gpsimd.indirect_dma_start(
                            out=er_t[:],
                            out_offset=None,
                            in_=er_dram[:],
                            in_offset=bass.IndirectOffsetOnAxis(ap=mem_t[:, 0:1], axis=0),
                        )
                    G = p2.tile([P, C * ROWP], BF16, name="G", tag="G")
                    if DBG_NO_GATHER:
                        nc.vector.memset(G[:], 0.0)
                    elif True:
                        pass
                    if not DBG_NO_GATHER:
                        # dma_gather ucode caps at 1024 indices (8 chunks)
                        MAXC = 8
                        for base, width, tb in ((0, CL, t_lo), (CL, CH, t_hi)):
                            done = 0
                            while done < width:
                                w = min(MAXC, width - done)
                                b = base + done
                                nc.gpsimd.dma_gather(
                                    out_ap=G[:, b * ROWP : (b + w) * ROWP].rearrange(
                                        "p (c r) -> p c r", c=w
                                    ),
                                    in_ap=tb[:, :],
                                    idxs_ap=idx_t[:, b * 8 : (b + w) * 8],
                                    num_idxs=w * P,
                                    num_idxs_reg=w * P,
                                    elem_size=ROWP,
                                )
                                done += w

                    if DBG_STAGE < -1:
                        of = p2.tile([P, D], F32, name="of", tag="of")
                        nc.vector.memset(of[:], 0.0)
                        nc.sync.dma_start(out=out[t * P : (t + 1) * P, :], in_=of[:])
                        off += C
                        continue
                    # ST[e, (j, d)] = (dstloc[e, j] == d)  edge-major one-hot
                    ST = p2.tile([P, C * P], BF16, name="ST", tag="ST")
                    if DBG_STAGE < 2:
                        nc.vector.memset(ST[:], 0.0)
                    if DBG_STAGE >= 2:
                     nc.vector.tensor_tensor(
                        out=ST[:].rearrange("p (c d) -> p c d", c=C),
                        in0=dl_t[:]
                        .rearrange("p (c one) -> p c one", one=1)
                        .to_broadcast([P, C, P]),
                        in1=iota_sb[:]
                        .rearrange("p (one d) -> p one d", one=1)
                        .to_broadcast([P, C, P]),
                        op=AL.is_equal,
                    )

                    # er gathered per edge: erg[e, h] = sum_d S[d, e] er_t[d, h]
                    erg_ps = ergps_pool.tile([P, C * H], F32, name="erg_ps")
                    if DBG_STAGE < 3:
                        for j in range(C):
                            nc.tensor.matmul(
                                erg_ps[:, j * H : (j + 1) * H],
                                lhsT=ST[:, j * P : (j + 1) * P],
                                rhs=er_t[:],
                                start=True,
                                stop=True,
                            )
                    if DBG_STAGE >= 3:
                     for j in range(C):
                        s_ps = sps_pool.tile([P, P], BF16, name="s_ps", tag="s_ps")
                        nc.tensor.transpose(
                            out=s_ps[:],
                            in_=ST[:, j * P : (j + 1) * P],
                            identity=ident_sb[:],
                        )
                        s_sb = p2s.tile([P, P], BF16, name="s_sb", tag="s_sb")
                        nc.any.tensor_copy(out=s_sb[:], in_=s_ps[:])
                        nc.tensor.matmul(
                            erg_ps[:, j * H : (j + 1) * H],
                            lhsT=s_sb[:],
                            rhs=er_t[:],
                            start=True,
                            stop=True,
                        )

                    # e_val = leaky_relu(el[src] + er[dst]); ex = exp(e_val)
                    ev = p2.tile([P, C * H], F32, name="ev", tag="ev")
                    if DBG_STAGE < 4:
                        nc.vector.tensor_copy(out=ev[:], in_=erg_ps[:])
                    if DBG_STAGE >= 4:
                     nc.vector.tensor_tensor(
                        out=ev[:].rearrange("p (c h) -> p c h", c=C),
                        in0=G[:].rearrange("p (c r) -> p c r", c=C)[:, :, HD : HD + H],
                        in1=erg_ps[:].rearrange("p (c h) -> p c h", c=C),
                        op=AL.add,
                    )
                    lrel = p2.tile([P, C * H], F32, name="lrel", tag="lrel")
                    nc.vector.scalar_tensor_tensor(
                        out=lrel[:],
                        in0=ev[:],
                        scalar=NEG_SLOPE,
                        in1=ev[:],
                        op0=AL.mult,
                        op1=AL.max,
                    )
                    exb = p2.tile([P, C * H], BF16, name="exb", tag="exb")
                    nc.scalar.activation(
                        out=exb[:], in_=lrel[:], func=mybir.ActivationFunctionType.Exp
                    )

                    # gs[e, h, :] = ex[e, h] * feat_src[e, h, :]
                    gs = p2.tile([P, C * HD], BF16, name="gs", tag="gs")
                    if DBG_STAGE < 5:
                        nc.vector.memset(gs[:], 0.0)
                    if DBG_STAGE >= 5:
                     nc.vector.tensor_tensor(
                        out=gs[:].rearrange("p (c h d) -> p c h d", c=C, h=H),
                        in0=G[:]
                        .rearrange("p (c r) -> p c r", c=C)[:, :, 0:HD]
                        .rearrange("p c (h d) -> p c h d", h=H),
                        in1=exb[:]
                        .rearrange("p (c h one) -> p c h one", h=H, one=1)
                        .to_broadcast([P, C, H, D]),
                        op=AL.mult,
                    )

                    out_ps = outps_pool.tile([P, HD], F32, name="out_ps")
                    den_ps = denps_pool.tile([P, H], F32, name="den_ps")
                    if DBG_STAGE < 1:
                        nc.tensor.matmul(den_ps[:], lhsT=ST[:, 0:P], rhs=exb[:, 0:H], start=True, stop=True)
                        nc.tensor.matmul(out_ps[:], lhsT=ST[:, 0:P], rhs=gs[:, 0:HD], start=True, stop=True)
                    if DBG_STAGE >= 1:
                     for j in range(C):
                        nc.tensor.matmul(
                            den_ps[:],
                            lhsT=ST[:, j * P : (j + 1) * P],
                            rhs=exb[:, j * H : (j + 1) * H],
                            start=(j == 0),
                            stop=(j == C - 1),
                        )
                        nc.tensor.matmul(
                            out_ps[:],
                            lhsT=ST[:, j * P : (j + 1) * P],
                            rhs=gs[:, j * HD : (j + 1) * HD],
                            start=(j == 0),
                            stop=(j == C - 1),
                        )

                    # epilogue: normalize, bias, tanh, mean over heads
                    if DBG_STAGE < 0:
                        of = p2.tile([P, D], F32, name="of", tag="of")
                        nc.vector.memset(of[:], 0.0)
                        nc.sync.dma_start(out=out[t * P : (t + 1) * P, :], in_=of[:])
                        off += C
                        continue
                    rd0 = p2s.tile([P, H], F32, name="rd0", tag="rd0")
                    nc.vector.tensor_scalar(
                        out=rd0[:], in0=den_ps[:], scalar1=1e-9, scalar2=None, op0=AL.max
                    )
                    rd = p2s.tile([P, H], F32, name="rd", tag="rd")
                    nc.vector.reciprocal(out=rd[:], in_=rd0[:])
                    nrm = p2.tile([P, HD], F32, name="nrm", tag="nrm")
                    nc.vector.tensor_tensor(
                        out=nrm[:].rearrange("p (h d) -> p h d", h=H),
                        in0=out_ps[:].rearrange("p (h d) -> p h d", h=H),
                        in1=rd[:]
                        .rearrange("p (h one) -> p h one", one=1)
                        .to_broadcast([P, H, D]),
                        op=AL.mult,
                    )
                    nb = p2.tile([P, HD], F32, name="nb", tag="nb")
                    nc.vector.tensor_tensor(
                        out=nb[:], in0=nrm[:], in1=bias_sb[:], op=AL.add
                    )
                    th = p2.tile([P, HD], F32, name="th", tag="th")
                    nc.scalar.activation(
                        out=th[:], in_=nb[:], func=mybir.ActivationFunctionType.Tanh
                    )
                    m1 = p2s.tile([P, D], F32, name="m1", tag="m1")
                    nc.vector.tensor_tensor(
                        out=m1[:], in0=th[:, 0:D], in1=th[:, D : 2 * D], op=AL.add
                    )
                    m2 = p2s.tile([P, D], F32, name="m2", tag="m2")
                    nc.vector.tensor_tensor(
                        out=m2[:],
                        in0=th[:, 2 * D : 3 * D],
                        in1=th[:, 3 * D : 4 * D],
                        op=AL.add,
                    )
                    m3 = p2s.tile([P, D], F32, name="m3", tag="m3")
                    nc.vector.tensor_tensor(out=m3[:], in0=m1[:], in1=m2[:], op=AL.add)
                    of = p2.tile([P, D], F32, name="of", tag="of")
                    nc.vector.tensor_scalar(
                        out=of[:], in0=m3[:], scalar1=0.25, scalar2=None, op0=AL.mult
                    )
                    nc.sync.dma_start(out=out[t * P : (t + 1) * P, :], in_=of[:])
                    off += C
    return nc


# --------------------------------------------------------------------------
# host entry
# --------------------------------------------------------------------------

def _make_static_inputs(h, W, attn_l, attn_r, bias):
    bf = ml_dtypes.bfloat16
    h_pad = np.zeros((N_PAD, IN_DIM), dtype=np.float32)
    h_pad[:N] = np.asarray(h, dtype=np.float32)
    hT = np.ascontiguousarray(h_pad.T).astype(bf)
    Wb = np.asarray(W, dtype=np.float32).astype(bf)
    WTb = np.ascontiguousarray(np.asarray(W, dtype=np.float32).T).astype(bf)
    ALRm = np.zeros((IN_DIM, 2 * H), dtype=np.float32)
    al = np.asarray(attn_l, dtype=np.float32)
    ar = np.asarray(attn_r, dtype=np.float32)
    for hh in range(H):
        ALRm[hh * D : (hh + 1) * D, hh] = al[hh]
        ALRm[hh * D : (hh + 1) * D, H + hh] = ar[hh]
    ALRm = ALRm.astype(bf)
    bias_rep = np.tile(np.asarray(bias, dtype=np.float32).reshape(1, HD), (P, 1))
    iota_rep = np.tile(np.arange(P, dtype=np.float32).reshape(1, P), (P, 1)).astype(bf)
    ident = np.eye(P, dtype=np.float32).astype(bf)
    return dict(
        hT=hT,
        Wb=Wb,
        WTb=WTb,
        ALR=ALRm,
        bias_rep=np.ascontiguousarray(bias_rep),
        iota_rep=np.ascontiguousarray(iota_rep),
        ident=np.ascontiguousarray(ident),
    )


def bench(nc, in_maps, n_iters=10):
    """Repeated-execution wall timing of the compiled SPMD kernel via PJRT.

    Returns (per_call_seconds_list, results_of_last_call)."""
    import time

    import jax
    from jax.sharding import Mesh, NamedSharding, PartitionSpec
    from jax.experimental.shard_map import shard_map

    from concourse import bass2jax, mybir as _mb

    bass2jax.install_neuronx_cc_hook()
    n_cores = len(in_maps)
    in_names, out_names, out_avals, zero_outs = [], [], [], []
    partition_name = nc.partition_id_tensor.name if nc.partition_id_tensor else None
    for alloc in nc.m.functions[0].allocations:
        if not isinstance(alloc, _mb.MemoryLocationSet):
            continue
        name = alloc.memorylocations[0].name
        if alloc.kind == "ExternalInput":
            if name != partition_name:
                in_names.append(name)
        elif alloc.kind == "ExternalOutput":
            out_names.append(name)
            shape = tuple(alloc.tensor_shape)
            dtype = _mb.dt.np(alloc.dtype)
            out_avals.append(jax.core.ShapedArray(shape, dtype))
            zero_outs.append(np.zeros(shape, dtype))
    n_params = len(in_names)
    all_in_names = in_names + out_names
    if partition_name is not None:
        all_in_names.append(partition_name)

    def _body(*args):
        operands = list(args)
        if partition_name is not None:
            operands.append(bass2jax.partition_id_tensor())
        outs = bass2jax._bass_exec_p.bind(
            *operands,
            out_avals=tuple(out_avals),
            in_names=tuple(all_in_names),
            out_names=tuple(out_names),
            lowering_input_output_aliases=(),
            sim_require_finite=True,
            sim_require_nnan=True,
            nc=nc,
        )
        return tuple(outs)

    devices = jax.devices()[:n_cores]
    mesh = Mesh(np.asarray(devices), ("core",))
    n_outs = len(out_names)
    sharded = jax.jit(
        shard_map(
            _body,
            mesh=mesh,
            in_specs=(PartitionSpec("core"),) * (n_params + n_outs),
            out_specs=(PartitionSpec("core"),) * n_outs,
            check_rep=False,
        ),
        keep_unused=True,
    )
    sh = NamedSharding(mesh, PartitionSpec("core"))
    concat_in = [
        jax.device_put(
            np.concatenate([np.asarray(in_maps[c][nm]) for c in range(n_cores)], 0), sh
        )
        for nm in in_names
    ]
    concat_zeros = [
        jax.device_put(np.zeros((n_cores * z.shape[0], *z.shape[1:]), z.dtype), sh)
        for z in zero_outs
    ]
    # warmup (compiles)
    outs = sharded(*concat_in, *concat_zeros)
    jax.block_until_ready(outs)
    times = []
    for _ in range(n_iters):
        t0 = time.perf_counter()
        outs = sharded(*concat_in, *concat_zeros)
        jax.block_until_ready(outs)
        times.append(time.perf_counter() - t0)
    results = [
        {
            nm: np.asarray(outs[i]).reshape(n_cores, *out_avals[i].shape)[c]
            for i, nm in enumerate(out_names)
        }
        for c in range(n_cores)
    ]
    return times, results


def kernel(h, W, attn_l, attn_r, bias, src, dst, trace=False):
    from concourse.bass_utils import run_bass_kernel_spmd

    aux, clo, chi = preprocess(src, dst)
    static = _make_static_inputs(h, W, attn_l, attn_r, bias)
    nc = build_kernel(N_PAD, TILES, clo, chi)
    nc.compile()  # bacc passes: matmul wait splitting, event sems, DCE
    in_maps = []
    host_tbl = {}
    if DBG_HOST_TABLE:
        bf = ml_dtypes.bfloat16
        hTf = static["hT"].astype(np.float32)
        Wf = static["Wb"].astype(np.float32)
        ALRf = static["ALR"].astype(np.float32)
        WALRf = (Wf @ ALRf).astype(bf).astype(np.float32)
        feat = (hTf.T @ Wf).astype(bf)
        elr = (hTf.T @ WALRf).astype(bf)
        tb = np.zeros((N_PAD, ROWP), dtype=bf)
        tb[:, 0:HD] = feat
        tb[:, HD : HD + 2 * H] = elr
        host_tbl = dict(
            t_lo_in=np.ascontiguousarray(tb[:HALF]),
            t_hi_in=np.ascontiguousarray(tb[HALF:]),
            er_in=np.ascontiguousarray(tb[:, HD + H : HD + 2 * H]),
        )
    for c in range(NC):
        m = dict(static)
        m.update(aux[c])
        m.update(host_tbl)
        in_maps.append(m)
    res = run_bass_kernel_spmd(nc, in_maps, core_ids=list(range(NC)), trace=False)
    out_full = np.zeros((N, D), dtype=np.float32)
    for c in range(NC):
        dev = res.results[c]["out"]  # [TILES*P, D]
        ids = aux[c]["member_ids"]  # [P, TILES]
        rows = ids.T.reshape(-1)  # row t*P+p  <->  ids[p, t]
        valid = rows < N
        out_full[rows[valid]] = dev[valid]
    kernel.last_nc = nc
    kernel.last_in_maps = in_maps
    kernel.last_aux = aux
    return out_full
